# revision 7
# baseline (speedup 1.0000x reference)
"""AttentionWithRoPE Trainium2 kernel (v2).

Sharding: hybrid batch x head tensor-parallel over 8 cores.
Core c handles batch b = c//4 and heads [4g, 4g+4) where g = c%4
(256 of the 1024 projection features). Each core computes its heads'
q/k/v projections, RoPE, full non-causal attention, and a partial
output projection; the host sums the 4 partial outputs per batch.

v2 design (vs v1): the ScalarE ACTIVATE cost is a flat ~1.1us per
instruction, so the 128 softmax exps are the hard floor; everything
else is arranged to keep ScalarE exp-only and gapless:
- RoPE runs entirely off ScalarE: t1 = ps*cos and u = ps*sin on DVE,
  the 32-row half-swap via 4 small SBUF->SBUF DMAs (gpsimd ring),
  one DVE add. Sign is baked into the sin table (+sin rows 0-31,
  -sin rows 32-63 per 64-block).
- Inputs stream position-slab-major so the first exp fires ~15us in
  (q slabs 0-1, k slab 0 first); window emission is paced in
  (head, kchunk, qhalf) half-windows with the qh=0 rounds first.
- Scores PSUM is double-buffered [128,1024] (tag s, 4 banks); the AV
  accumulators are per-(head, qhalf) [65,1024] tiles (tag o, 4 banks).
  Projection/vp PSUM shares the s-tag rotation.
- AV lags the exp producer by 6-8 half-windows; the final head runs at
  lag 2 so the tail is short. Head 3 normalizes via an indicator
  matmul; heads 0-2 via the DRAM-bounce partition broadcast.
- Output projection starts right after the last norm with a few PE
  warmers covering the norm latency; output DMA round-robins over the
  sync/scalar/gpsimd rings.
"""

import sys

if "/opt/trn_rl_repo" not in sys.path:
    sys.path.insert(0, "/opt/trn_rl_repo")

import numpy as np
import ml_dtypes

B, L, DIM, H = 2, 2048, 1024, 16
HD = 64
NCORES = 8
FPC = 256          # features per core
NHC = 4            # heads per core
KCH = DIM // 128   # 8 contraction chunks of 128
NSLAB = 4          # position slabs of 512
BF = ml_dtypes.bfloat16

_PROG_CACHE = {}


# --------------------------------------------------------------------------
# workarounds: the walrus in this container encodes at most ONE semaphore
# wait per instruction; split extra waits onto preceding same-engine NOPs.
# --------------------------------------------------------------------------
def _install_patches():
    import concourse.tile as tile_mod
    import bass_rust as _br
    from concourse.vector_clock import ScopedClock

    if getattr(tile_mod, "_ant_wait_split_installed", False):
        return

    def _split_multi_waits(nc, ordered):
        for bb_name, insts in ordered.items():
            new_list = []
            for inst in insts:
                si = getattr(inst, "sync_info", None)
                ws = list(si.on_wait) if (si is not None and si.on_wait) else []
                if len(ws) > 1:
                    try:
                        eng = inst.engine
                        for extra in ws[:-1]:
                            nop = _br.InstNoOp(name=f"I-wsplit-{nc.next_id()}")
                            nop.engine = eng
                            nop.sync_info = _br.SyncInfo(
                                on_wait=[extra], on_update=[]
                            )
                            new_list.append(nop)
                        inst.sync_info = _br.SyncInfo(
                            on_wait=[ws[-1]], on_update=list(si.on_update or [])
                        )
                    except Exception:
                        pass
                new_list.append(inst)
            ordered[bb_name] = new_list

    _orig_lower = tile_mod.TileContext._lower_ordered_insts

    def _patched_lower(self, ordered):
        _split_multi_waits(self.nc, ordered)
        return _orig_lower(self, ordered)

    def _patched_dab(self, tick_clock, wait_clock):
        probe = self.nc.sync.nop(nofuse=True)
        wait_clock.add_sem_waits(
            probe.ins, ScopedClock({None: tick_clock.global_clock})
        )
        si = probe.ins.sync_info
        w = list(si.on_wait) if si and si.on_wait else []
        if len(w) > 1:
            probe.ins.sync_info = _br.SyncInfo(
                on_wait=w[:1], on_update=list(si.on_update or [])
            )
            for i in range(1, len(w)):
                n2 = self.nc.sync.nop(nofuse=True)
                n2.ins.sync_info = _br.SyncInfo(on_wait=[w[i]], on_update=[])
        self.nc.sync.drain()
        self.nc.all_engine_barrier()
        assert self.sems is not None
        popped = self.nc._tile_sem_poison_stack.pop()
        assert popped is self._sem_poison
        self.nc.clear_and_free_semaphores(list(self.sems.allocated().values()))
        self.nc.all_engine_barrier()

    tile_mod.TileContext._lower_ordered_insts = _patched_lower
    tile_mod.TileContext._drain_and_barrier = _patched_dab
    tile_mod._ant_wait_split_installed = True


# --------------------------------------------------------------------------
# device program
# --------------------------------------------------------------------------
def _build_program(with_bias):
    _install_patches()
    import concourse.bass as bass
    import concourse.tile as tile
    from concourse import mybir

    f32 = mybir.dt.float32
    bf16 = mybir.dt.bfloat16
    EXP = mybir.ActivationFunctionType.Exp

    nc = bass.Bass()

    xq = nc.dram_tensor("xq", [KCH, 128, L], bf16, kind="ExternalInput")
    xk = nc.dram_tensor("xk", [KCH, 128, L], bf16, kind="ExternalInput")
    xv = nc.dram_tensor("xv", [KCH, 128, L], bf16, kind="ExternalInput")
    wq = nc.dram_tensor("wq", [KCH, 128, FPC], bf16, kind="ExternalInput")
    wk = nc.dram_tensor("wk", [KCH, 128, FPC], bf16, kind="ExternalInput")
    wv = nc.dram_tensor("wv", [KCH, 128, FPC], bf16, kind="ExternalInput")
    wo = nc.dram_tensor("wo", [2, 128, DIM], bf16, kind="ExternalInput")
    if with_bias:
        bq = nc.dram_tensor("bq", [1, FPC], bf16, kind="ExternalInput")
        bk = nc.dram_tensor("bk", [1, FPC], bf16, kind="ExternalInput")
        bv = nc.dram_tensor("bv", [1, FPC], bf16, kind="ExternalInput")
    cosT = nc.dram_tensor("cosT", [NSLAB, 128, 512], bf16, kind="ExternalInput")
    sinT = nc.dram_tensor("sinT", [NSLAB, 128, 512], bf16, kind="ExternalInput")
    outT = nc.dram_tensor("outT", [KCH, 128, L], f32, kind="ExternalOutput")

    with tile.TileContext(nc) as tc:
        from contextlib import ExitStack

        with ExitStack() as ctx:
            const = ctx.enter_context(tc.tile_pool(name="const", bufs=1))
            psum = ctx.enter_context(
                tc.tile_pool(name="psum", bufs=1, space="PSUM")
            )
            wp = ctx.enter_context(tc.tile_pool(name="wp", bufs=1))
            inp = ctx.enter_context(tc.tile_pool(name="inp", bufs=16))
            tabp = ctx.enter_context(tc.tile_pool(name="tabp", bufs=1))
            ropep = ctx.enter_context(tc.tile_pool(name="ropep", bufs=1))
            tmpp = ctx.enter_context(tc.tile_pool(name="tmpp", bufs=3))
            vpp = ctx.enter_context(tc.tile_pool(name="vpp", bufs=16))
            ptp = ctx.enter_context(tc.tile_pool(name="ptp", bufs=8))
            oallp = ctx.enter_context(tc.tile_pool(name="oallp", bufs=1))
            onn = ctx.enter_context(tc.tile_pool(name="onn", bufs=1))
            outp = ctx.enter_context(tc.tile_pool(name="outp", bufs=2))
            dramp = ctx.enter_context(
                tc.tile_pool(name="dramp", bufs=2, space="DRAM")
            )

            # ---- ScalarE exp table preload (hides the ~2.7us table load) --
            pre_in = const.tile([1, 8], f32)
            nc.vector.memset(pre_in, 0.0)
            pre_out = const.tile([1, 8], bf16)
            nc.scalar.activation(pre_out, pre_in, EXP, scale=1.0)

            # ---- indicator matrix for the tail head's 1/Z broadcast ------
            ind4 = const.tile([97, 4 * 64], f32)
            nc.vector.memset(ind4, 0.0)
            for r in range(4):
                nc.vector.memset(ind4[r * 32: r * 32 + 1,
                                      r * 64: (r + 1) * 64], 1.0)

            bias_sb = {}
            ones_bf = None
            if with_bias:
                for name, dram in (("bq", bq), ("bk", bk), ("bv", bv)):
                    bt = const.tile([1, FPC], bf16, name=f"{name}_sb")
                    nc.sync.dma_start(out=bt, in_=dram[:, :])
                    bias_sb[name] = bt
                ones_bf = const.tile([1, 512], bf16)
                nc.vector.memset(ones_bf, 1.0)

            # ---- input DMA, arrival-ordered -----------------------------
            wq_sb = wp.tile([128, KCH * FPC], bf16, name="wq_sb")
            wk_sb = wp.tile([128, KCH * FPC], bf16, name="wk_sb")
            wv_sb = wp.tile([128, KCH * FPC], bf16, name="wv_sb")
            xch = {}
            for t in ("q", "k", "v"):
                for kc in range(KCH):
                    xch[t, kc] = inp.tile(
                        [128, L], bf16, tag=f"in{t}", bufs=8, name=f"x_{t}{kc}"
                    )

            def dma_w(w_sb, dram):
                for kc in range(KCH):
                    nc.sync.dma_start(
                        out=w_sb[:, kc * FPC:(kc + 1) * FPC], in_=dram[kc]
                    )

            def dma_xslab(t, dram, s):
                for kc in range(KCH):
                    nc.sync.dma_start(
                        out=xch[t, kc][:, s * 512:(s + 1) * 512],
                        in_=dram[kc][:, s * 512:(s + 1) * 512],
                    )

            dma_w(wq_sb, wq)
            dma_xslab("q", xq, 0)
            dma_xslab("q", xq, 1)
            dma_w(wk_sb, wk)
            dma_xslab("k", xk, 0)
            dma_w(wv_sb, wv)
            dma_xslab("v", xv, 0)
            dma_xslab("k", xk, 1)
            dma_xslab("v", xv, 1)
            dma_xslab("k", xk, 2)
            dma_xslab("k", xk, 3)
            dma_xslab("q", xq, 2)
            dma_xslab("q", xq, 3)
            dma_xslab("v", xv, 2)
            dma_xslab("v", xv, 3)

            # tables + wo on the scalar ring
            cos_sb = tabp.tile([128, L], bf16, name="cos_sb")
            sin_sb = tabp.tile([128, L], bf16, name="sin_sb")
            for s in range(NSLAB):
                nc.scalar.dma_start(
                    out=cos_sb[:, s * 512:(s + 1) * 512], in_=cosT[s]
                )
                nc.scalar.dma_start(
                    out=sin_sb[:, s * 512:(s + 1) * 512], in_=sinT[s]
                )
            wo_sb = []
            for t_i in range(2):
                w = wp.tile([128, DIM], bf16, name=f"wo_sb{t_i}")
                nc.scalar.dma_start(out=w, in_=wo[t_i])
                wo_sb.append(w)

            # ---- rope'd q/k tiles ---------------------------------------
            ropes = {}
            for tens in ("q", "k"):
                for t_i in range(2):
                    ropes[tens, t_i] = ropep.tile(
                        [128, L], bf16, name=f"rope_{tens}{t_i}"
                    )

            # ---- unit emitters ------------------------------------------
            def proj_unit(tens, t_i, s):
                """Project (tens, t_i) features for position slab s, then
                RoPE on DVE with the swap via gpsimd SBUF->SBUF DMAs."""
                w_sb = wq_sb if tens == "q" else wk_sb
                bn = "bq" if tens == "q" else "bk"
                ps = psum.tile([128, 512], f32, tag="s", bufs=2,
                               name=f"ps_{tens}{t_i}{s}")
                lo = t_i * 128
                for kc in range(KCH):
                    nc.tensor.matmul(
                        ps,
                        w_sb[:, kc * FPC + lo: kc * FPC + lo + 128],
                        xch[tens, kc][:, s * 512:(s + 1) * 512],
                        start=(kc == 0),
                        stop=(kc == KCH - 1 and not with_bias),
                    )
                if with_bias:
                    nc.tensor.matmul(
                        ps, bias_sb[bn][:, lo: lo + 128], ones_bf[:, :512],
                        start=False, stop=True,
                    )
                c = cos_sb[:, s * 512:(s + 1) * 512]
                sn = sin_sb[:, s * 512:(s + 1) * 512]
                t1 = tmpp.tile([128, 512], f32, tag="tmp", name=f"t1_{tens}{t_i}{s}")
                u = tmpp.tile([128, 512], f32, tag="tmp", name=f"u_{tens}{t_i}{s}")
                usw = tmpp.tile([128, 512], f32, tag="tmp",
                                name=f"usw_{tens}{t_i}{s}")
                nc.vector.tensor_mul(t1, ps, c)
                nc.vector.tensor_mul(u, ps, sn)
                for b in (0, 64):
                    nc.gpsimd.dma_start(out=usw[b: b + 32], in_=u[b + 32: b + 64])
                    nc.gpsimd.dma_start(out=usw[b + 32: b + 64], in_=u[b: b + 32])
                nc.vector.tensor_add(
                    ropes[tens, t_i][:, s * 512:(s + 1) * 512], t1, usw
                )

            vp_tiles = {}

            def vp_unit(st):
                vps = psum.tile([128, FPC], f32, tag="s", bufs=2,
                                name=f"vps{st}")
                for kc in range(KCH):
                    nc.tensor.matmul(
                        vps,
                        xch["v", kc][:, st * 128:(st + 1) * 128],
                        wv_sb[:, kc * FPC:(kc + 1) * FPC],
                        start=(kc == 0),
                        stop=(kc == KCH - 1 and not with_bias),
                    )
                if with_bias:
                    nc.tensor.matmul(
                        vps, ones_bf[:, :128], bias_sb["bv"],
                        start=False, stop=True,
                    )
                vt = vpp.tile([128, NHC * 65], bf16, tag="vp", name=f"vp{st}")
                vtr = vt.rearrange("p (h c) -> p h c", c=65)
                nc.vector.memset(vtr[:, :, 64], 1.0)
                nc.vector.tensor_copy(
                    vtr[:, :, 0:64], vps.rearrange("p (h c) -> p h c", c=64)
                )
                vp_tiles[st] = vt

            # ---- window half machinery ----------------------------------
            # half g = (h, kc, qh): S^T chunk [128 kpos, 1024 q] + exp
            pts = {}

            def win(h, kc, qh):
                t_i = h // 2
                off = (h % 2) * 64
                kr, qr = ropes["k", t_i], ropes["q", t_i]
                sps = psum.tile([128, 1024], f32, tag="s", bufs=2,
                                name=f"sps_{h}_{kc}_{qh}")
                for qs in range(2):
                    nc.tensor.matmul(
                        sps[:, qs * 512:(qs + 1) * 512],
                        kr[off: off + 64, kc * 128:(kc + 1) * 128],
                        qr[off: off + 64,
                           qh * 1024 + qs * 512: qh * 1024 + (qs + 1) * 512],
                        start=True, stop=True,
                    )
                pt = ptp.tile([128, 1024], bf16, tag="pt",
                              name=f"pt_{h}_{kc}_{qh}")
                nc.scalar.activation(pt, sps, EXP, scale=0.125)
                pts[h, kc, qh] = pt

            o65 = {}
            ou = {}
            zb4 = {}

            def av_step(h, kc, qh):
                if kc == 0:
                    o65[h, qh] = psum.tile([65, 1024], f32, tag="o", bufs=2,
                                           name=f"o65_{h}_{qh}")
                lh = vp_tiles[kc][:, h * 65:(h + 1) * 65]
                pt = pts.pop((h, kc, qh))
                for qs in range(2):
                    nc.tensor.matmul(
                        o65[h, qh][:, qs * 512:(qs + 1) * 512],
                        lh, pt[:, qs * 512:(qs + 1) * 512],
                        start=(kc == 0), stop=(kc == 15),
                    )
                if kc == 15:
                    gather(h, qh)

            def gather(h, qh):
                """Free the o65 accumulator: copy out numerators and the
                1/Z source rows."""
                if qh == 0:
                    z = onn.tile([97, 512], f32, tag="zb4", bufs=2,
                                 name=f"zb4_{h}")
                    if h == NHC - 1:
                        nc.vector.memset(z, 1.0)
                    zb4[h] = z
                o = o65.pop((h, qh))
                for qs in range(2):
                    r = 2 * qh + qs
                    ot = onn.tile([64, 512], f32, tag="ou", bufs=6,
                                  name=f"ou_{h}_{r}")
                    nc.vector.tensor_copy(ot, o[0:64, qs * 512:(qs + 1) * 512])
                    ou[h, r] = ot
                    nc.vector.tensor_copy(
                        zb4[h][r * 32: r * 32 + 1, :],
                        o[64:65, qs * 512:(qs + 1) * 512],
                    )

            def norm_head(h):
                t_i = h // 2
                off = (h % 2) * 64
                zi4 = onn.tile([97, 512], f32, tag="zi4", bufs=1,
                               name=f"zi4_{h}")
                nc.vector.reciprocal(zi4, zb4[h])
                if h == NHC - 1:
                    for pair in range(2):
                        zbp = psum.tile([64, 1024], f32, tag="s", bufs=2,
                                        name=f"zbp_{h}_{pair}")
                        for q in range(2):
                            r = pair * 2 + q
                            nc.tensor.matmul(
                                zbp[:, q * 512:(q + 1) * 512],
                                ind4[:, r * 64:(r + 1) * 64], zi4,
                                start=True, stop=True,
                            )
                        for q in range(2):
                            r = pair * 2 + q
                            nc.vector.tensor_mul(
                                oall[t_i][off: off + 64,
                                          r * 512:(r + 1) * 512],
                                ou.pop((h, r)),
                                zbp[:, q * 512:(q + 1) * 512],
                            )
                else:
                    zd = dramp.tile([4, 512], f32, tag="zd", name=f"zd_{h}")
                    for r in range(4):
                        nc.sync.dma_start(
                            out=zd[r: r + 1, :],
                            in_=zi4[r * 32: r * 32 + 1, :],
                        )
                    for r in range(4):
                        zb = onn.tile([64, 512], f32, tag="zb", bufs=2,
                                      name=f"zb_{h}_{r}")
                        zsrc = zd[r: r + 1, :]
                        bc = bass.AP(
                            tensor=zsrc.tensor, offset=zsrc.offset,
                            ap=[[0, 64]] + list(zsrc.ap)[1:],
                        )
                        nc.gpsimd.dma_start(out=zb, in_=bc)
                        nc.vector.tensor_mul(
                            oall[t_i][off: off + 64, r * 512:(r + 1) * 512],
                            ou.pop((h, r)), zb,
                        )

            oall = []
            for t_i in range(2):
                oall.append(
                    oallp.tile([128, L], bf16, name=f"oall{t_i}")
                )

            # ---- emission schedule --------------------------------------
            proj_unit("q", 0, 0)
            proj_unit("q", 0, 1)
            proj_unit("k", 0, 0)

            fills = [
                (2, lambda: vp_unit(0)),
                (4, lambda: proj_unit("k", 0, 1)),
                (5, lambda: vp_unit(1)),
                (6, lambda: vp_unit(2)),
                (8, lambda: vp_unit(3)),
                (10, lambda: proj_unit("k", 0, 2)),
                (11, lambda: vp_unit(4)),
                (12, lambda: vp_unit(5)),
                (14, lambda: proj_unit("k", 0, 3)),
                (15, lambda: vp_unit(6)),
                (16, lambda: vp_unit(7)),
                (18, lambda: vp_unit(8)),
                (20, lambda: vp_unit(9)),
                (22, lambda: proj_unit("q", 0, 2)),
                (23, lambda: vp_unit(10)),
                (24, lambda: proj_unit("q", 0, 3)),
                (25, lambda: vp_unit(11)),
                (26, lambda: vp_unit(12)),
                (28, lambda: vp_unit(13)),
                (30, lambda: vp_unit(14)),
                (32, lambda: vp_unit(15)),
                (42, lambda: proj_unit("k", 1, 0)),
                (44, lambda: proj_unit("k", 1, 1)),
                (46, lambda: proj_unit("k", 1, 2)),
                (48, lambda: proj_unit("k", 1, 3)),
                (50, lambda: proj_unit("q", 1, 0)),
                (52, lambda: proj_unit("q", 1, 1)),
                (54, lambda: proj_unit("q", 1, 2)),
                (56, lambda: proj_unit("q", 1, 3)),
            ]

            halves = []
            for kc in range(16):          # P0: heads 0,1 qh0 interleaved
                for h in (0, 1):
                    halves.append((h, kc, 0))
            for kc in range(16):          # P1: heads 0,1 qh1
                for h in (0, 1):
                    halves.append((h, kc, 1))
            for kc in range(16):          # P2: heads 2,3 qh0
                for h in (2, 3):
                    halves.append((h, kc, 0))
            for kc in range(16):          # P3a: head 2 qh1
                halves.append((2, kc, 1))
            for kc in range(16):          # P3b: head 3 qh1
                halves.append((3, kc, 1))

            av_cursor = 0
            norms_done = set()

            def drain_av(upto):
                nonlocal av_cursor
                while av_cursor <= upto:
                    h, kc, qh = halves[av_cursor]
                    av_step(h, kc, qh)
                    av_cursor += 1
                    if kc == 15 and qh == 1 and h < NHC - 1:
                        norm_head(h)
                        norms_done.add(h)

            fi = 0
            for g, (h, kc, qh) in enumerate(halves):
                while fi < len(fills) and fills[fi][0] <= g:
                    fills[fi][1]()
                    fi += 1
                win(h, kc, qh)
                lag = 6 if g < 112 else 2
                drain_av(g - lag)
            while fi < len(fills):
                fills[fi][1]()
                fi += 1
            drain_av(len(halves) - 1)

            # ---- tail: warmers + last norm + output projection ----------
            warm = psum.tile([128, 1024], f32, tag="s", bufs=2, name="warm")
            for _ in range(40):
                nc.tensor.matmul(
                    warm[:, 0:256], wo_sb[0][:, 0:128], oall[0][:, 0:256],
                    start=True, stop=True,
                )
            norm_head(NHC - 1)

            qrr = [nc.sync, nc.scalar, nc.gpsimd]
            for od in range(KCH):
                for half in range(2):
                    hof = half * 1024
                    cps = psum.tile([128, 1024], f32, tag="s", bufs=2,
                                    name=f"cps_{od}_{half}")
                    for t_i in range(2):
                        for qs in range(2):
                            nc.tensor.matmul(
                                cps[:, qs * 512:(qs + 1) * 512],
                                wo_sb[t_i][:, od * 128:(od + 1) * 128],
                                oall[t_i][:, hof + qs * 512: hof + (qs + 1) * 512],
                                start=(t_i == 0), stop=(t_i == 1),
                            )
                    ot = outp.tile([128, 1024], f32, tag="ot", bufs=2,
                                   name=f"ot_{od}_{half}")
                    if (od * 2 + half) % 2 == 0:
                        nc.vector.tensor_copy(ot, cps)
                    else:
                        nc.scalar.copy(ot, cps)
                    qrr[(od * 2 + half) % 3].dma_start(
                        out=outT[od][:, hof: hof + 1024], in_=ot
                    )

    return nc


def _get_program(with_bias):
    key = ("nc", with_bias)
    if key not in _PROG_CACHE:
        _PROG_CACHE[key] = _build_program(with_bias)
    return _PROG_CACHE[key]


# --------------------------------------------------------------------------
# host-side helpers
# --------------------------------------------------------------------------
def _rope_tables():
    inv = (
        1.0 / (10000.0 ** (np.arange(HD // 2, dtype=np.float32) * 2.0 / HD))
    ).astype(np.float32)
    ang = np.arange(L, dtype=np.float32)[:, None] * inv[None, :]  # [L, 32]
    cosL = np.cos(ang).astype(np.float32).T  # [32, L]
    sinL = np.sin(ang).astype(np.float32).T
    blk_c = np.concatenate([cosL, cosL], axis=0)           # [64, L]
    blk_s = np.concatenate([sinL, -sinL], axis=0)          # sign-baked
    cos128 = np.ascontiguousarray(np.concatenate([blk_c, blk_c], axis=0))
    sin128 = np.ascontiguousarray(np.concatenate([blk_s, blk_s], axis=0))
    cos4 = np.ascontiguousarray(
        cos128.reshape(128, NSLAB, 512).transpose(1, 0, 2)
    ).astype(BF)
    sin4 = np.ascontiguousarray(
        sin128.reshape(128, NSLAB, 512).transpose(1, 0, 2)
    ).astype(BF)
    return cos4, sin4


def _wchunks(Mc):
    """[256, 1024] weight rows -> transposed chunked [8, 128, 256] bf16."""
    return np.ascontiguousarray(Mc.T.astype(BF)).reshape(KCH, 128, FPC)


def kernel(q, k, v, Wq, bq, Wk, bk, Wv, bv, Wo, bo, _trace=False):
    q, k, v = (np.asarray(a, dtype=np.float32) for a in (q, k, v))
    Wq, Wk, Wv, Wo = (np.asarray(a, dtype=np.float32) for a in (Wq, Wk, Wv, Wo))
    bq, bk, bv, bo = (np.asarray(a, dtype=np.float32) for a in (bq, bk, bv, bo))

    with_bias = bool(np.any(bq) or np.any(bk) or np.any(bv))
    nc = _get_program(with_bias)
    from concourse.bass_utils import run_bass_kernel_spmd

    cos_t, sin_t = _rope_tables()
    xt = {}
    for b in range(B):
        for nm, arr in (("q", q), ("k", k), ("v", v)):
            xt[nm, b] = np.ascontiguousarray(arr[b].T.astype(BF)).reshape(
                KCH, 128, L
            )

    in_maps = []
    for c in range(NCORES):
        b, g = c // 4, c % 4
        fs = slice(g * FPC, (g + 1) * FPC)
        m = {
            "xq": xt["q", b], "xk": xt["k", b], "xv": xt["v", b],
            "wq": _wchunks(Wq[fs, :]),
            "wk": _wchunks(Wk[fs, :]),
            "wv": _wchunks(Wv[fs, :]),
            "wo": np.ascontiguousarray(Wo[:, fs].T.astype(BF)).reshape(
                2, 128, DIM
            ),
            "cosT": cos_t, "sinT": sin_t,
        }
        if with_bias:
            m["bq"] = bq[fs].astype(BF).reshape(1, FPC)
            m["bk"] = bk[fs].astype(BF).reshape(1, FPC)
            m["bv"] = bv[fs].astype(BF).reshape(1, FPC)
        in_maps.append(m)

    res = run_bass_kernel_spmd(
        nc, in_maps, core_ids=list(range(NCORES)), trace=_trace
    )
    out = np.zeros((B, L, DIM), np.float32)
    for c in range(NCORES):
        b = c // 4
        oT = np.asarray(res.results[c]["outT"]).reshape(DIM, L)
        out[b] += oT.T
    out += bo[None, None, :]
    if _trace:
        return out, res
    return out


# revision 8
# speedup vs baseline: 1.2118x; 1.2118x over previous
"""AttentionWithRoPE Trainium2 kernel (v2).

Sharding: hybrid batch x head tensor-parallel over 8 cores.
Core c handles batch b = c//4 and heads [4g, 4g+4) where g = c%4
(256 of the 1024 projection features). Each core computes its heads'
q/k/v projections, RoPE, full non-causal attention, and a partial
output projection; the host sums the 4 partial outputs per batch.

v2 design (vs v1): the ScalarE ACTIVATE cost is a flat ~1.1us per
instruction, so the 128 softmax exps are the hard floor; everything
else is arranged to keep ScalarE exp-only and gapless:
- RoPE runs entirely off ScalarE: t1 = ps*cos and u = ps*sin on DVE,
  the 32-row half-swap via 4 small SBUF->SBUF DMAs (gpsimd ring),
  one DVE add. Sign is baked into the sin table (+sin rows 0-31,
  -sin rows 32-63 per 64-block).
- Inputs stream position-slab-major so the first exp fires ~15us in
  (q slabs 0-1, k slab 0 first); window emission is paced in
  (head, kchunk, qhalf) half-windows with the qh=0 rounds first.
- Scores PSUM is double-buffered [128,1024] (tag s, 4 banks); the AV
  accumulators are per-(head, qhalf) [65,1024] tiles (tag o, 4 banks).
  Projection/vp PSUM shares the s-tag rotation.
- AV lags the exp producer by 6-8 half-windows; the final head runs at
  lag 2 so the tail is short. Head 3 normalizes via an indicator
  matmul; heads 0-2 via the DRAM-bounce partition broadcast.
- Output projection starts right after the last norm with a few PE
  warmers covering the norm latency; output DMA round-robins over the
  sync/scalar/gpsimd rings.
"""

import sys

if "/opt/trn_rl_repo" not in sys.path:
    sys.path.insert(0, "/opt/trn_rl_repo")

import numpy as np
import ml_dtypes

B, L, DIM, H = 2, 2048, 1024, 16
HD = 64
NCORES = 8
FPC = 256          # features per core
NHC = 4            # heads per core
KCH = DIM // 128   # 8 contraction chunks of 128
NSLAB = 2          # position slabs of 1024 (2KB DMA lines)
BF = ml_dtypes.bfloat16

_PROG_CACHE = {}


# --------------------------------------------------------------------------
# workarounds: the walrus in this container encodes at most ONE semaphore
# wait per instruction; split extra waits onto preceding same-engine NOPs.
# --------------------------------------------------------------------------
def _install_patches():
    import concourse.tile as tile_mod
    import bass_rust as _br
    from concourse.vector_clock import ScopedClock

    if getattr(tile_mod, "_ant_wait_split_installed", False):
        return

    def _split_multi_waits(nc, ordered):
        for bb_name, insts in ordered.items():
            new_list = []
            for inst in insts:
                si = getattr(inst, "sync_info", None)
                ws = list(si.on_wait) if (si is not None and si.on_wait) else []
                if len(ws) > 1:
                    try:
                        eng = inst.engine
                        for extra in ws[:-1]:
                            nop = _br.InstNoOp(name=f"I-wsplit-{nc.next_id()}")
                            nop.engine = eng
                            nop.sync_info = _br.SyncInfo(
                                on_wait=[extra], on_update=[]
                            )
                            new_list.append(nop)
                        inst.sync_info = _br.SyncInfo(
                            on_wait=[ws[-1]], on_update=list(si.on_update or [])
                        )
                    except Exception:
                        pass
                new_list.append(inst)
            ordered[bb_name] = new_list

    _orig_lower = tile_mod.TileContext._lower_ordered_insts

    def _patched_lower(self, ordered):
        _split_multi_waits(self.nc, ordered)
        return _orig_lower(self, ordered)

    def _patched_dab(self, tick_clock, wait_clock):
        probe = self.nc.sync.nop(nofuse=True)
        wait_clock.add_sem_waits(
            probe.ins, ScopedClock({None: tick_clock.global_clock})
        )
        si = probe.ins.sync_info
        w = list(si.on_wait) if si and si.on_wait else []
        if len(w) > 1:
            probe.ins.sync_info = _br.SyncInfo(
                on_wait=w[:1], on_update=list(si.on_update or [])
            )
            for i in range(1, len(w)):
                n2 = self.nc.sync.nop(nofuse=True)
                n2.ins.sync_info = _br.SyncInfo(on_wait=[w[i]], on_update=[])
        self.nc.sync.drain()
        self.nc.all_engine_barrier()
        assert self.sems is not None
        popped = self.nc._tile_sem_poison_stack.pop()
        assert popped is self._sem_poison
        self.nc.clear_and_free_semaphores(list(self.sems.allocated().values()))
        self.nc.all_engine_barrier()

    tile_mod.TileContext._lower_ordered_insts = _patched_lower
    tile_mod.TileContext._drain_and_barrier = _patched_dab
    tile_mod._ant_wait_split_installed = True


# --------------------------------------------------------------------------
# device program
# --------------------------------------------------------------------------
def _build_program(with_bias):
    _install_patches()
    import concourse.bass as bass
    import concourse.tile as tile
    from concourse import mybir

    f32 = mybir.dt.float32
    bf16 = mybir.dt.bfloat16
    EXP = mybir.ActivationFunctionType.Exp

    nc = bass.Bass()

    xq = nc.dram_tensor("xq", [KCH, 128, L], bf16, kind="ExternalInput")
    xk = nc.dram_tensor("xk", [KCH, 128, L], bf16, kind="ExternalInput")
    xv = nc.dram_tensor("xv", [KCH, 128, L], bf16, kind="ExternalInput")
    wq = nc.dram_tensor("wq", [KCH, 128, FPC], bf16, kind="ExternalInput")
    wk = nc.dram_tensor("wk", [KCH, 128, FPC], bf16, kind="ExternalInput")
    wv = nc.dram_tensor("wv", [KCH, 128, FPC], bf16, kind="ExternalInput")
    wo = nc.dram_tensor("wo", [2, 128, DIM], bf16, kind="ExternalInput")
    if with_bias:
        bq = nc.dram_tensor("bq", [1, FPC], bf16, kind="ExternalInput")
        bk = nc.dram_tensor("bk", [1, FPC], bf16, kind="ExternalInput")
        bv = nc.dram_tensor("bv", [1, FPC], bf16, kind="ExternalInput")
    cosT = nc.dram_tensor("cosT", [NSLAB, 128, 1024], bf16, kind="ExternalInput")
    sinT = nc.dram_tensor("sinT", [NSLAB, 128, 1024], bf16, kind="ExternalInput")
    outT = nc.dram_tensor("outT", [KCH, 128, L], f32, kind="ExternalOutput")

    with tile.TileContext(nc) as tc:
        from contextlib import ExitStack

        with ExitStack() as ctx:
            const = ctx.enter_context(tc.tile_pool(name="const", bufs=1))
            psum = ctx.enter_context(
                tc.tile_pool(name="psum", bufs=1, space="PSUM")
            )
            wp = ctx.enter_context(tc.tile_pool(name="wp", bufs=1))
            inp = ctx.enter_context(tc.tile_pool(name="inp", bufs=16))
            tabp = ctx.enter_context(tc.tile_pool(name="tabp", bufs=1))
            ropep = ctx.enter_context(tc.tile_pool(name="ropep", bufs=1))
            tmpp = ctx.enter_context(tc.tile_pool(name="tmpp", bufs=3))
            vpp = ctx.enter_context(tc.tile_pool(name="vpp", bufs=16))
            ptp = ctx.enter_context(tc.tile_pool(name="ptp", bufs=8))
            oallp = ctx.enter_context(tc.tile_pool(name="oallp", bufs=1))
            onn = ctx.enter_context(tc.tile_pool(name="onn", bufs=1))
            outp = ctx.enter_context(tc.tile_pool(name="outp", bufs=2))
            dramp = ctx.enter_context(
                tc.tile_pool(name="dramp", bufs=2, space="DRAM")
            )

            # ---- ScalarE exp table preload (hides the ~2.7us table load) --
            pre_in = const.tile([1, 8], f32)
            nc.vector.memset(pre_in, 0.0)
            pre_out = const.tile([1, 8], bf16)
            nc.scalar.activation(pre_out, pre_in, EXP, scale=1.0)

            # ---- indicator matrix for the tail head's 1/Z broadcast ------
            ind4 = const.tile([97, 4 * 64], f32)
            nc.vector.memset(ind4, 0.0)
            for r in range(4):
                nc.vector.memset(ind4[r * 32: r * 32 + 1,
                                      r * 64: (r + 1) * 64], 1.0)

            bias_sb = {}
            ones_bf = None
            if with_bias:
                for name, dram in (("bq", bq), ("bk", bk), ("bv", bv)):
                    bt = const.tile([1, FPC], bf16, name=f"{name}_sb")
                    nc.sync.dma_start(out=bt, in_=dram[:, :])
                    bias_sb[name] = bt
                ones_bf = const.tile([1, 512], bf16)
                nc.vector.memset(ones_bf, 1.0)

            # ---- input DMA, arrival-ordered -----------------------------
            wq_sb = wp.tile([128, KCH * FPC], bf16, name="wq_sb")
            wk_sb = wp.tile([128, KCH * FPC], bf16, name="wk_sb")
            wv_sb = wp.tile([128, KCH * FPC], bf16, name="wv_sb")
            xch = {}
            for t in ("q", "k", "v"):
                for kc in range(KCH):
                    xch[t, kc] = inp.tile(
                        [128, L], bf16, tag=f"in{t}", bufs=8, name=f"x_{t}{kc}"
                    )

            def dma_w(eng, w_sb, dram):
                for kc in range(KCH):
                    eng.dma_start(
                        out=w_sb[:, kc * FPC:(kc + 1) * FPC], in_=dram[kc]
                    )

            def dma_xslab(eng, t, dram, s):
                for kc in range(KCH):
                    eng.dma_start(
                        out=xch[t, kc][:, s * 1024:(s + 1) * 1024],
                        in_=dram[kc][:, s * 1024:(s + 1) * 1024],
                    )

            cos_sb = tabp.tile([128, L], bf16, name="cos_sb")
            sin_sb = tabp.tile([128, L], bf16, name="sin_sb")

            # sync ring: q then k (the exp-critical stream)
            dma_w(nc.sync, wq_sb, wq)
            dma_xslab(nc.sync, "q", xq, 0)
            dma_w(nc.sync, wk_sb, wk)
            dma_xslab(nc.sync, "k", xk, 0)
            dma_xslab(nc.sync, "k", xk, 1)
            dma_xslab(nc.sync, "q", xq, 1)
            # scalar ring: v projection inputs + rope tables + wo
            dma_w(nc.scalar, wv_sb, wv)
            for s in range(NSLAB):
                nc.scalar.dma_start(
                    out=cos_sb[:, s * 1024:(s + 1) * 1024], in_=cosT[s]
                )
                nc.scalar.dma_start(
                    out=sin_sb[:, s * 1024:(s + 1) * 1024], in_=sinT[s]
                )
            dma_xslab(nc.scalar, "v", xv, 0)
            dma_xslab(nc.scalar, "v", xv, 1)
            wo_sb = []
            for t_i in range(2):
                w = wp.tile([128, DIM], bf16, name=f"wo_sb{t_i}")
                nc.scalar.dma_start(out=w, in_=wo[t_i])
                wo_sb.append(w)

            # ---- rope'd q/k tiles ---------------------------------------
            ropes = {}
            for tens in ("q", "k"):
                for t_i in range(2):
                    ropes[tens, t_i] = ropep.tile(
                        [128, L], bf16, name=f"rope_{tens}{t_i}"
                    )

            # ---- unit emitters ------------------------------------------
            def proj_unit(tens, t_i, s):
                """Project (tens, t_i) features for position slab s (1024
                cols), then RoPE on DVE with the 32-row half-swap done by
                4 small SBUF->SBUF DMAs on the gpsimd ring."""
                w_sb = wq_sb if tens == "q" else wk_sb
                bn = "bq" if tens == "q" else "bk"
                ps = psum.tile([128, 1024], f32, tag="s", bufs=2,
                               name=f"ps_{tens}{t_i}{s}")
                lo = t_i * 128
                for kc in range(KCH):
                    for qs in range(2):
                        nc.tensor.matmul(
                            ps[:, qs * 512:(qs + 1) * 512],
                            w_sb[:, kc * FPC + lo: kc * FPC + lo + 128],
                            xch[tens, kc][:, s * 1024 + qs * 512:
                                          s * 1024 + (qs + 1) * 512],
                            start=(kc == 0),
                            stop=(kc == KCH - 1 and not with_bias),
                        )
                if with_bias:
                    for qs in range(2):
                        nc.tensor.matmul(
                            ps[:, qs * 512:(qs + 1) * 512],
                            bias_sb[bn][:, lo: lo + 128], ones_bf[:, :512],
                            start=False, stop=True,
                        )
                c = cos_sb[:, s * 1024:(s + 1) * 1024]
                sn = sin_sb[:, s * 1024:(s + 1) * 1024]
                t1 = tmpp.tile([128, 1024], bf16, tag="tmp",
                               name=f"t1_{tens}{t_i}{s}")
                u = tmpp.tile([128, 1024], bf16, tag="tmp",
                              name=f"u_{tens}{t_i}{s}")
                usw = tmpp.tile([128, 1024], bf16, tag="tmp",
                                name=f"usw_{tens}{t_i}{s}")
                nc.vector.tensor_mul(t1, ps, c)
                nc.vector.tensor_mul(u, ps, sn)
                for b in (0, 64):
                    nc.gpsimd.dma_start(out=usw[b: b + 32], in_=u[b + 32: b + 64])
                    nc.gpsimd.dma_start(out=usw[b + 32: b + 64], in_=u[b: b + 32])
                nc.vector.tensor_add(
                    ropes[tens, t_i][:, s * 1024:(s + 1) * 1024], t1, usw
                )

            vp_tiles = {}

            def vp_unit(st):
                vps = psum.tile([128, FPC], f32, tag="s", bufs=2,
                                name=f"vps{st}")
                for kc in range(KCH):
                    nc.tensor.matmul(
                        vps,
                        xch["v", kc][:, st * 128:(st + 1) * 128],
                        wv_sb[:, kc * FPC:(kc + 1) * FPC],
                        start=(kc == 0),
                        stop=(kc == KCH - 1 and not with_bias),
                    )
                if with_bias:
                    nc.tensor.matmul(
                        vps, ones_bf[:, :128], bias_sb["bv"],
                        start=False, stop=True,
                    )
                vt = vpp.tile([128, NHC * 65], bf16, tag="vp", name=f"vp{st}")
                vtr = vt.rearrange("p (h c) -> p h c", c=65)
                nc.vector.memset(vtr[:, :, 64], 1.0)
                nc.vector.tensor_copy(
                    vtr[:, :, 0:64], vps.rearrange("p (h c) -> p h c", c=64)
                )
                vp_tiles[st] = vt

            # ---- window half machinery ----------------------------------
            # half g = (h, kc, qh): S^T chunk [128 kpos, 1024 q] + exp
            pts = {}

            def win(h, kc, qh):
                t_i = h // 2
                off = (h % 2) * 64
                kr, qr = ropes["k", t_i], ropes["q", t_i]
                sps = psum.tile([128, 1024], f32, tag="s", bufs=2,
                                name=f"sps_{h}_{kc}_{qh}")
                for qs in range(2):
                    nc.tensor.matmul(
                        sps[:, qs * 512:(qs + 1) * 512],
                        kr[off: off + 64, kc * 128:(kc + 1) * 128],
                        qr[off: off + 64,
                           qh * 1024 + qs * 512: qh * 1024 + (qs + 1) * 512],
                        start=True, stop=True,
                    )
                pt = ptp.tile([128, 1024], bf16, tag="pt",
                              name=f"pt_{h}_{kc}_{qh}")
                nc.scalar.activation(pt, sps, EXP, scale=0.125)
                pts[h, kc, qh] = pt

            o65 = {}
            ou = {}
            zb4 = {}

            def av_step(h, kc, qh):
                if kc == 0:
                    o65[h, qh] = psum.tile([65, 1024], f32, tag="o", bufs=2,
                                           name=f"o65_{h}_{qh}")
                lh = vp_tiles[kc][:, h * 65:(h + 1) * 65]
                pt = pts.pop((h, kc, qh))
                for qs in range(2):
                    nc.tensor.matmul(
                        o65[h, qh][:, qs * 512:(qs + 1) * 512],
                        lh, pt[:, qs * 512:(qs + 1) * 512],
                        start=(kc == 0), stop=(kc == 15),
                    )
                if kc == 15:
                    gather(h, qh)

            def gather(h, qh):
                """Free the o65 accumulator: copy out numerators and the
                1/Z source rows."""
                if qh == 0:
                    z = onn.tile([97, 512], f32, tag="zb4", bufs=2,
                                 name=f"zb4_{h}")
                    if h == NHC - 1:
                        nc.vector.memset(z, 1.0)
                    zb4[h] = z
                o = o65.pop((h, qh))
                for qs in range(2):
                    r = 2 * qh + qs
                    ot = onn.tile([64, 512], f32, tag="ou", bufs=6,
                                  name=f"ou_{h}_{r}")
                    nc.vector.tensor_copy(ot, o[0:64, qs * 512:(qs + 1) * 512])
                    ou[h, r] = ot
                    nc.vector.tensor_copy(
                        zb4[h][r * 32: r * 32 + 1, :],
                        o[64:65, qs * 512:(qs + 1) * 512],
                    )

            def norm_head(h):
                t_i = h // 2
                off = (h % 2) * 64
                zi4 = onn.tile([97, 512], f32, tag="zi4", bufs=1,
                               name=f"zi4_{h}")
                nc.vector.reciprocal(zi4, zb4[h])
                if h == NHC - 1:
                    for pair in range(2):
                        zbp = psum.tile([64, 1024], f32, tag="s", bufs=2,
                                        name=f"zbp_{h}_{pair}")
                        for q in range(2):
                            r = pair * 2 + q
                            nc.tensor.matmul(
                                zbp[:, q * 512:(q + 1) * 512],
                                ind4[:, r * 64:(r + 1) * 64], zi4,
                                start=True, stop=True,
                            )
                        for q in range(2):
                            r = pair * 2 + q
                            nc.vector.tensor_mul(
                                oall[t_i][off: off + 64,
                                          r * 512:(r + 1) * 512],
                                ou.pop((h, r)),
                                zbp[:, q * 512:(q + 1) * 512],
                            )
                else:
                    zd = dramp.tile([4, 512], f32, tag="zd", name=f"zd_{h}")
                    for r in range(4):
                        nc.sync.dma_start(
                            out=zd[r: r + 1, :],
                            in_=zi4[r * 32: r * 32 + 1, :],
                        )
                    for r in range(4):
                        zb = onn.tile([64, 512], f32, tag="zb", bufs=2,
                                      name=f"zb_{h}_{r}")
                        zsrc = zd[r: r + 1, :]
                        bc = bass.AP(
                            tensor=zsrc.tensor, offset=zsrc.offset,
                            ap=[[0, 64]] + list(zsrc.ap)[1:],
                        )
                        nc.gpsimd.dma_start(out=zb, in_=bc)
                        nc.vector.tensor_mul(
                            oall[t_i][off: off + 64, r * 512:(r + 1) * 512],
                            ou.pop((h, r)), zb,
                        )

            oall = []
            for t_i in range(2):
                oall.append(
                    oallp.tile([128, L], bf16, name=f"oall{t_i}")
                )

            # ---- emission schedule --------------------------------------
            proj_unit("q", 0, 0)
            proj_unit("k", 0, 0)

            fills = [
                (2, lambda: vp_unit(0)),
                (4, lambda: vp_unit(1)),
                (6, lambda: vp_unit(2)),
                (8, lambda: proj_unit("k", 0, 1)),
                (9, lambda: vp_unit(3)),
                (11, lambda: vp_unit(4)),
                (13, lambda: vp_unit(5)),
                (15, lambda: vp_unit(6)),
                (17, lambda: vp_unit(7)),
                (18, lambda: vp_unit(8)),
                (20, lambda: vp_unit(9)),
                (22, lambda: proj_unit("q", 0, 1)),
                (23, lambda: vp_unit(10)),
                (25, lambda: vp_unit(11)),
                (26, lambda: vp_unit(12)),
                (28, lambda: vp_unit(13)),
                (30, lambda: vp_unit(14)),
                (32, lambda: vp_unit(15)),
                (44, lambda: proj_unit("k", 1, 0)),
                (48, lambda: proj_unit("k", 1, 1)),
                (52, lambda: proj_unit("q", 1, 0)),
                (56, lambda: proj_unit("q", 1, 1)),
            ]

            halves = []
            for kc in range(16):          # P0: heads 0,1 qh0 interleaved
                for h in (0, 1):
                    halves.append((h, kc, 0))
            for kc in range(16):          # P1: heads 0,1 qh1
                for h in (0, 1):
                    halves.append((h, kc, 1))
            for kc in range(16):          # P2: heads 2,3 qh0
                for h in (2, 3):
                    halves.append((h, kc, 0))
            for kc in range(16):          # P3a: head 2 qh1
                halves.append((2, kc, 1))
            for kc in range(16):          # P3b: head 3 qh1
                halves.append((3, kc, 1))

            av_cursor = 0
            norms_done = set()

            def drain_av(upto):
                nonlocal av_cursor
                while av_cursor <= upto:
                    h, kc, qh = halves[av_cursor]
                    av_step(h, kc, qh)
                    av_cursor += 1
                    if kc == 15 and qh == 1 and h < NHC - 1:
                        norm_head(h)
                        norms_done.add(h)

            fi = 0
            for g, (h, kc, qh) in enumerate(halves):
                while fi < len(fills) and fills[fi][0] <= g:
                    fills[fi][1]()
                    fi += 1
                win(h, kc, qh)
                lag = 6 if g < 112 else 2
                drain_av(g - lag)
            while fi < len(fills):
                fills[fi][1]()
                fi += 1
            drain_av(len(halves) - 1)

            # ---- tail: warmers + last norm + output projection ----------
            warm = psum.tile([128, 1024], f32, tag="s", bufs=2, name="warm")
            for _ in range(40):
                nc.tensor.matmul(
                    warm[:, 0:256], wo_sb[0][:, 0:128], oall[0][:, 0:256],
                    start=True, stop=True,
                )
            norm_head(NHC - 1)

            qrr = [nc.sync, nc.scalar, nc.gpsimd]
            for od in range(KCH):
                for half in range(2):
                    hof = half * 1024
                    cps = psum.tile([128, 1024], f32, tag="s", bufs=2,
                                    name=f"cps_{od}_{half}")
                    for t_i in range(2):
                        for qs in range(2):
                            nc.tensor.matmul(
                                cps[:, qs * 512:(qs + 1) * 512],
                                wo_sb[t_i][:, od * 128:(od + 1) * 128],
                                oall[t_i][:, hof + qs * 512: hof + (qs + 1) * 512],
                                start=(t_i == 0), stop=(t_i == 1),
                            )
                    ot = outp.tile([128, 1024], f32, tag="ot", bufs=2,
                                   name=f"ot_{od}_{half}")
                    if (od * 2 + half) % 2 == 0:
                        nc.vector.tensor_copy(ot, cps)
                    else:
                        nc.scalar.copy(ot, cps)
                    qrr[(od * 2 + half) % 3].dma_start(
                        out=outT[od][:, hof: hof + 1024], in_=ot
                    )

    return nc


def _get_program(with_bias):
    key = ("nc", with_bias)
    if key not in _PROG_CACHE:
        _PROG_CACHE[key] = _build_program(with_bias)
    return _PROG_CACHE[key]


# --------------------------------------------------------------------------
# host-side helpers
# --------------------------------------------------------------------------
def _rope_tables():
    inv = (
        1.0 / (10000.0 ** (np.arange(HD // 2, dtype=np.float32) * 2.0 / HD))
    ).astype(np.float32)
    ang = np.arange(L, dtype=np.float32)[:, None] * inv[None, :]  # [L, 32]
    cosL = np.cos(ang).astype(np.float32).T  # [32, L]
    sinL = np.sin(ang).astype(np.float32).T
    blk_c = np.concatenate([cosL, cosL], axis=0)           # [64, L]
    blk_s = np.concatenate([sinL, -sinL], axis=0)          # sign-baked
    cos128 = np.ascontiguousarray(np.concatenate([blk_c, blk_c], axis=0))
    sin128 = np.ascontiguousarray(np.concatenate([blk_s, blk_s], axis=0))
    cos4 = np.ascontiguousarray(
        cos128.reshape(128, NSLAB, 1024).transpose(1, 0, 2)
    ).astype(BF)
    sin4 = np.ascontiguousarray(
        sin128.reshape(128, NSLAB, 1024).transpose(1, 0, 2)
    ).astype(BF)
    return cos4, sin4


def _wchunks(Mc):
    """[256, 1024] weight rows -> transposed chunked [8, 128, 256] bf16."""
    return np.ascontiguousarray(Mc.T.astype(BF)).reshape(KCH, 128, FPC)


def kernel(q, k, v, Wq, bq, Wk, bk, Wv, bv, Wo, bo, _trace=False):
    q, k, v = (np.asarray(a, dtype=np.float32) for a in (q, k, v))
    Wq, Wk, Wv, Wo = (np.asarray(a, dtype=np.float32) for a in (Wq, Wk, Wv, Wo))
    bq, bk, bv, bo = (np.asarray(a, dtype=np.float32) for a in (bq, bk, bv, bo))

    with_bias = bool(np.any(bq) or np.any(bk) or np.any(bv))
    nc = _get_program(with_bias)
    from concourse.bass_utils import run_bass_kernel_spmd

    cos_t, sin_t = _rope_tables()
    xt = {}
    for b in range(B):
        for nm, arr in (("q", q), ("k", k), ("v", v)):
            xt[nm, b] = np.ascontiguousarray(arr[b].T.astype(BF)).reshape(
                KCH, 128, L
            )

    in_maps = []
    for c in range(NCORES):
        b, g = c // 4, c % 4
        fs = slice(g * FPC, (g + 1) * FPC)
        m = {
            "xq": xt["q", b], "xk": xt["k", b], "xv": xt["v", b],
            "wq": _wchunks(Wq[fs, :]),
            "wk": _wchunks(Wk[fs, :]),
            "wv": _wchunks(Wv[fs, :]),
            "wo": np.ascontiguousarray(Wo[:, fs].T.astype(BF)).reshape(
                2, 128, DIM
            ),
            "cosT": cos_t, "sinT": sin_t,
        }
        if with_bias:
            m["bq"] = bq[fs].astype(BF).reshape(1, FPC)
            m["bk"] = bk[fs].astype(BF).reshape(1, FPC)
            m["bv"] = bv[fs].astype(BF).reshape(1, FPC)
        in_maps.append(m)

    res = run_bass_kernel_spmd(
        nc, in_maps, core_ids=list(range(NCORES)), trace=_trace
    )
    out = np.zeros((B, L, DIM), np.float32)
    for c in range(NCORES):
        b = c // 4
        oT = np.asarray(res.results[c]["outT"]).reshape(DIM, L)
        out[b] += oT.T
    out += bo[None, None, :]
    if _trace:
        return out, res
    return out
